# revision 40
# baseline (speedup 1.0000x reference)
"""AHGCRU (hypergraph-conv GRU) Trainium2 kernel.

Data-parallel over batch: B=16 -> 2 batch elements per NeuronCore (8 cores),
graph/params replicated, no collectives.

Host precompute collapses the N->M->N hypergraph aggregation into one dense
(N,N) matrix A2 = diag(Dinv) @ adj @ diag(Binv) @ S.T  (adjacency depends only
on nodevec/edgevec, so it is identical for every batch element and timestep),
then rank-128-truncates it via SVD (A2 ~= Ur @ Vr): the spectral tail is far
below the bf16 noise floor, and the factored aggregation
pre += Ur @ (Vr @ hl) costs 4x fewer PE columns than the dense product.

Device layout: channels-on-partitions, nodes on the free dim, both local batch
elements side by side: (C, 2048) = [b0 nodes 0:1024 | b1 nodes 1024:2048]
(N=1000 padded to 1024).  LayerNorm over channels is done with PE assists:
  - mean-centering via a projection-matrix GEMM  P = diag(gamma) @ (I - 1/C)
  - variance via a weighted ones-matrix GEMM     G[k,m] = (1/C)/gamma_k^2
  - rstd = exp(-0.5*ln(var+eps)) on ScalarE (ln/exp share one act table).
Sigmoid/tanh are computed as exp (ScalarE, same natural_log_exp table -> no
ACT_TABLE_LOAD per step) + fast custom-DVE reciprocal; relu runs on DVE
(tensor_scalar_max) so the ScalarE chain is ln/exp/square/copies only.
Every PSUM tile is one [128,512] bank keyed per (batch, half) so chain
stages pipeline tile-granularly; agg banks are emitted interleaved with the
previous tile's LN chain to keep PE continuously busy (p-state ramp).
State channels sit at partitions 0:64, a constant ones-row at 64 (also the
bias row of the bias-augmented 1x1 conv), and x_t channels at 65:97.
"""

import os
import sys

import numpy as np

for _p in ("/opt/trn_rl_repo", "/opt/pypackages"):
    if os.path.isdir(_p) and _p not in sys.path:
        sys.path.insert(0, _p)

B, N, F_IN, T = 16, 1000, 32, 12
HID = 64
OUT = 64
M = 500
EMB = 16
EPS = 1e-5

NCORES = 8
BL = B // NCORES          # 2 batch elements per core
NP = 1024                 # padded node count
NC = BL * NP              # 2048 free columns
NCHUNK = NP // 128        # 8 source chunks of 128 nodes per batch element


# --------------------------------------------------------------------------
# host-side preprocessing
# --------------------------------------------------------------------------

def _to_bf16(a):
    import ml_dtypes  # noqa: PLC0415

    return np.asarray(a, dtype=np.float32).astype(ml_dtypes.bfloat16)


def _host_prep(inputs):
    """Build all device-side constant tensors (shared across cores)."""
    f64 = np.float64
    nodevec = inputs["nodevec"].astype(f64)
    edgevec = inputs["edgevec"].astype(f64)

    DE = np.tanh(2.0 * nodevec)                     # (N, EMB)
    EE = np.tanh(2.0 * edgevec)                     # (M, EMB)
    adj = np.maximum(np.tanh(2.0 * (DE @ EE.T)), 0.0)   # (N, M)
    S = (adj > 0).astype(f64)
    Bsum = S.sum(0)
    Binv = np.where(Bsum > 0, 1.0 / np.maximum(Bsum, 1e-30), 0.0)   # (M,)
    Dsum = adj.sum(1)
    Dinv = np.where(Dsum > 0, 1.0 / np.maximum(Dsum, 1e-30), 0.0)   # (N,)

    A2 = (Dinv[:, None] * adj * Binv[None, :]) @ S.T     # (N, N): out = A2 @ hl
    # rank-R factorization A2 ~= Ur @ Vr: the SVD tail is far below the
    # bf16 noise floor (rel err 6.598e-3 at R=128 vs 6.580e-3 dense), and
    # the factored aggregation costs 48R output columns vs 24576 dense
    R = int(os.environ.get("A2_RANK", "128"))
    _u, _s, _vt = np.linalg.svd(A2)
    Ur = _u[:, :R] * _s[:R]                              # (N, R)
    Vr = _vt[:R]                                         # (R, N)
    VrT = np.zeros((NP, R), f64)
    VrT[:N] = Vr.T
    VrT = np.ascontiguousarray(VrT).reshape(NCHUNK, 128, R)
    UrT = np.zeros((R, NP), f64)
    UrT[:, :N] = Ur.T

    # channel reorder: xs rows were [xt 0:32 | state 32:96]; device buffer
    # uses [state 0:64 | ones 64 | xt 65:97] so the ones row also serves the
    # bias-augmented 1x1 conv (rhs rows 0:65)
    def reorder_aug(w, b_res, b_lin):
        # (96, C) weights -> (97, C) with the bias row at position 64
        w = np.asarray(w, f64)
        bias = np.asarray(b_res, f64) + np.asarray(b_lin, f64)
        return np.concatenate([w[32:96], bias[None, :], w[0:32]], axis=0)

    def reorder_zero(w):
        # (96, C) weights -> (97, C) with a zero row at position 64
        w = np.asarray(w, f64)
        z = np.zeros((1, w.shape[1]), f64)
        return np.concatenate([w[32:96], z, w[0:32]], axis=0)

    # gate output channels reordered to [r | z] so that r sits at partition
    # base 0 (walrus requires equal start partitions for DVE SB inputs)
    zperm = np.concatenate([np.arange(64, 128), np.arange(0, 64)])
    w_lin_g = reorder_zero(inputs["w_lin_g"])[:, zperm]       # (97, 128)
    wr_g = reorder_aug(inputs["w_res_g"], inputs["b_res_g"],
                       inputs["b_lin_g"])[:, zperm]
    w_lin_c = reorder_zero(inputs["w_lin_c"])                 # (97, 64)
    wr_c = reorder_aug(inputs["w_res_c"], inputs["b_res_c"], inputs["b_lin_c"])
    convw_aug = np.concatenate(
        [np.asarray(inputs["conv_w"], f64),
         np.asarray(inputs["conv_b"], f64)[None, :]], axis=0)  # (65, 64)

    def center_mats(gamma, C):
        g = np.asarray(gamma, f64)
        P = np.diag(g) @ (np.eye(C) - np.ones((C, C)) / C)    # cent = P @ pre
        gsq = np.where(g != 0, g * g, 1.0)
        w = (1.0 / C) / gsq                                   # var weights
        G = np.repeat(w[:, None], C, axis=1)                  # (C, C)
        return P, G

    Pg, Gg = center_mats(np.asarray(inputs["ln_g_w"], f64)[zperm], 2 * HID)
    Pc, Gc = center_mats(inputs["ln_c_w"], HID)               # (64, 64)

    def blockdiag(Ab, Bb):
        Z = np.zeros((Ab.shape[0] + Bb.shape[0], Ab.shape[1] + Bb.shape[1]), f64)
        Z[: Ab.shape[0], : Ab.shape[1]] = Ab
        Z[Ab.shape[0]:, Ab.shape[1]:] = Bb
        return Z

    Pcb = blockdiag(Pc, Pc)                                   # (128, 128)
    Gcb = blockdiag(Gc, Gc)

    consts = {
        "vrt": _to_bf16(VrT),                 # (8, 128, 128)
        "urt": _to_bf16(UrT),                 # (128, 1024)
        "wlin_g": _to_bf16(w_lin_g),          # (97, 128)
        "wres_g": _to_bf16(wr_g),             # (97, 128)
        "wlin_c": _to_bf16(w_lin_c),          # (97, 64)
        "wres_c": _to_bf16(wr_c),             # (97, 64)
        # lhsT for cent = P @ pre must be P.T (out = lhsT.T @ rhs)
        "pg": _to_bf16(Pg.T),                 # (128, 128)
        "gg": _to_bf16(Gg),                   # (128, 128) symmetric-by-rows
        "pcb": _to_bf16(Pcb.T),               # (128, 128)
        "gcb": _to_bf16(Gcb),
        "convw": _to_bf16(convw_aug),         # (65, 64)
        "ln_g_b": np.asarray(inputs["ln_g_b"], np.float32)[zperm],
        "ln_c_b": np.asarray(inputs["ln_c_b"], np.float32),
    }
    return consts


def _host_x(inputs):
    """x (B, N, F_IN, T) -> per-core (T, 32, 2048) bf16, channel-transposed."""
    x = np.asarray(inputs["x"], np.float32)
    xt = x.transpose(3, 2, 0, 1)                      # (T, F_IN, B, N)
    xp = np.zeros((T, F_IN, B, NP), np.float32)
    xp[:, :, :, :N] = xt
    shards = []
    for c in range(NCORES):
        sl = xp[:, :, c * BL:(c + 1) * BL, :].reshape(T, F_IN, NC)
        shards.append(_to_bf16(sl))
    return shards


# --------------------------------------------------------------------------
# device program
# --------------------------------------------------------------------------

def _patch_tile_drain():
    """walrus in this toolchain rejects >~2 sync-waits on one instruction;
    Tile's kernel-tail drain accumulates one wait per dangling semaphore.
    Split them across single-wait nofuse nops on the sync engine."""
    import concourse.mybir as mybir  # noqa: PLC0415
    from concourse.tile import TileContext  # noqa: PLC0415
    from concourse.vector_clock import ScopedClock  # noqa: PLC0415

    if getattr(TileContext, "_drain_waits_patched", False):
        return

    def _drain_and_barrier(self, tick_clock, wait_clock):
        collector = self.nc.sync.nop(nofuse=True, hint="tail_wait_0")
        wait_clock.add_sem_waits(
            collector.ins, ScopedClock({None: tick_clock.global_clock})
        )
        si = collector.ins.sync_info
        waits = list(si.on_wait) if si and si.on_wait else []
        if len(waits) > 1:
            collector.ins.sync_info = mybir.SyncInfo(
                on_wait=[waits[0]], on_update=list(si.on_update or [])
            )
            for k, w in enumerate(waits[1:]):
                extra = self.nc.sync.nop(nofuse=True, hint=f"tail_wait_{k + 1}")
                extra.ins.sync_info = mybir.SyncInfo(on_wait=[w], on_update=[])
        self.nc.sync.drain()
        self.nc.all_engine_barrier()
        popped = self.nc._tile_sem_poison_stack.pop()
        assert popped is self._sem_poison
        self.nc.clear_and_free_semaphores(list(self.sems.allocated().values()))
        self.nc.all_engine_barrier()

    TileContext._drain_and_barrier = _drain_and_barrier

    # Split >MAX_WAITS sem-waits on any scheduled instruction onto preceding
    # nofuse nops on the same engine (same-engine program order preserves
    # the wait semantics exactly).
    MAX_WAITS = int(os.environ.get("BASS_MAX_INST_WAITS", "1"))
    orig_lower = TileContext._lower_ordered_insts

    def _lower_ordered_insts(self, ordered):
        for bb_name, insts in ordered.items():
            out = []
            for inst in insts:
                si = inst.sync_info
                waits = list(si.on_wait) if si and si.on_wait else []
                if len(waits) > MAX_WAITS:
                    excess = waits[:-MAX_WAITS]
                    keep = waits[-MAX_WAITS:]
                    for j in range(0, len(excess), MAX_WAITS):
                        nop = mybir.InstNoOp(
                            name=self.nc.get_next_instruction_name(),
                            ins=[], outs=[], engine=inst.engine,
                        )
                        nop.bass_nofuse = True
                        nop.sync_info = mybir.SyncInfo(
                            on_wait=excess[j:j + MAX_WAITS], on_update=[]
                        )
                        out.append(nop)
                    inst.sync_info = mybir.SyncInfo(
                        on_wait=keep, on_update=list(si.on_update or [])
                    )
                out.append(inst)
            insts[:] = out
        return orig_lower(self, ordered)

    TileContext._lower_ordered_insts = _lower_ordered_insts
    TileContext._drain_waits_patched = True


def _build_bass(beta_g_nonzero, beta_c_nonzero):
    import concourse.bass as bass  # noqa: PLC0415
    import concourse.mybir as mybir  # noqa: PLC0415
    from concourse.tile import TileContext  # noqa: PLC0415

    _patch_tile_drain()

    fp32 = mybir.dt.float32
    bf16 = mybir.dt.bfloat16
    AF = mybir.ActivationFunctionType

    nc = bass.Bass()

    _reps = int(os.environ.get("WORK_REPS", "1"))
    rep_tag = nc.declare_dram_parameter("rep_tag", [1, 8 * _reps], fp32,
                                        isOutput=False)
    xT = nc.declare_dram_parameter("xT", [T, F_IN, NC], bf16, isOutput=False)
    vrt_d = nc.declare_dram_parameter("vrt", [NCHUNK, 128, 128], bf16, isOutput=False)
    urt_d = nc.declare_dram_parameter("urt", [128, NP], bf16, isOutput=False)
    wlin_g_d = nc.declare_dram_parameter("wlin_g", [97, 128], bf16, isOutput=False)
    wres_g_d = nc.declare_dram_parameter("wres_g", [97, 128], bf16, isOutput=False)
    wlin_c_d = nc.declare_dram_parameter("wlin_c", [97, 64], bf16, isOutput=False)
    wres_c_d = nc.declare_dram_parameter("wres_c", [97, 64], bf16, isOutput=False)
    pg_d = nc.declare_dram_parameter("pg", [128, 128], bf16, isOutput=False)
    gg_d = nc.declare_dram_parameter("gg", [128, 128], bf16, isOutput=False)
    pcb_d = nc.declare_dram_parameter("pcb", [128, 128], bf16, isOutput=False)
    gcb_d = nc.declare_dram_parameter("gcb", [128, 128], bf16, isOutput=False)
    convw_d = nc.declare_dram_parameter("convw", [65, 64], bf16, isOutput=False)
    out_d = nc.declare_dram_parameter("out", [T, OUT, NC], bf16, isOutput=True)

    with TileContext(nc) as tc:
        with (
            tc.tile_pool(name="const", bufs=1) as cpool,
            tc.tile_pool(name="state", bufs=1) as spool,
            tc.tile_pool(name="work", bufs=2) as wpool,
            tc.tile_pool(name="psA", bufs=1, space="PSUM") as psA,
            tc.tile_pool(name="psB", bufs=1, space="PSUM") as psB,
        ):
            # ---- constants into SBUF -------------------------------------
            vrt = cpool.tile([128, NCHUNK, 128], bf16, tag="vrt")
            for s in range(NCHUNK):
                nc.sync.dma_start(vrt[:, s, :], vrt_d[s])
            urt = cpool.tile([128, NP], bf16, tag="urt")
            nc.sync.dma_start(urt[:], urt_d[:])
            wlin_g = cpool.tile([97, 128], bf16, tag="wlg")
            nc.sync.dma_start(wlin_g[:], wlin_g_d[:])
            wres_g = cpool.tile([97, 128], bf16, tag="wrg")
            nc.sync.dma_start(wres_g[:], wres_g_d[:])
            wlin_c = cpool.tile([97, 64], bf16, tag="wlc")
            nc.sync.dma_start(wlin_c[:], wlin_c_d[:])
            wres_c = cpool.tile([97, 64], bf16, tag="wrc")
            nc.sync.dma_start(wres_c[:], wres_c_d[:])
            pg = cpool.tile([128, 128], bf16, tag="pg")
            nc.sync.dma_start(pg[:], pg_d[:])
            gg = cpool.tile([128, 128], bf16, tag="gg")
            nc.sync.dma_start(gg[:], gg_d[:])
            pcb = cpool.tile([128, 128], bf16, tag="pcb")
            nc.sync.dma_start(pcb[:], pcb_d[:])
            gcb = cpool.tile([128, 128], bf16, tag="gcb")
            nc.sync.dma_start(gcb[:], gcb_d[:])
            convw = cpool.tile([65, 64], bf16, tag="convw")
            nc.sync.dma_start(convw[:], convw_d[:])
            epsv = cpool.tile([128, 1], fp32, tag="epsv")
            nc.vector.memset(epsv[:], EPS)
            rtag = cpool.tile([1, 8 * _reps], fp32, tag="rtag")
            nc.sync.dma_start(rtag[:], rep_tag[:])

            # ---- persistent state buffers --------------------------------
            # per-(b, j) chunk tiles, 512 node cols each, so dependencies are
            # tracked at chunk granularity and the serial chain pipelines
            xsA = [[spool.tile([97, 512], bf16, tag=f"xsA{b}{j}",
                               name=f"xsA{b}{j}") for j in range(2)]
                   for b in range(BL)]
            xsB = [[spool.tile([97, 512], bf16, tag=f"xsB{b}{j}",
                               name=f"xsB{b}{j}") for j in range(2)]
                   for b in range(BL)]
            xcb = [[spool.tile([97, 512], bf16, tag=f"xc{b}{j}",
                               name=f"xcb{b}{j}") for j in range(2)]
                   for b in range(BL)]
            for b in range(BL):
                for j in range(2):
                    nc.vector.memset(xsA[b][j][64:65, :], 1.0)
                    nc.vector.memset(xsB[b][j][64:65, :], 1.0)
                    nc.vector.memset(xcb[b][j][64:65, :], 1.0)
                    nc.vector.memset(xsA[b][j][0:64, :], 0.0)   # h_0 = 0

            for b in range(BL):
                for j in range(2):
                    nc.vector.memset(xsB[b][j][0:64, :], 0.0)

            WORK_REPS = int(os.environ.get("WORK_REPS", "1"))

            for ti, t in enumerate([tt % T for tt in range(T * WORK_REPS)]):
                xs = xsA if ti % 2 == 0 else xsB
                xs_next = xsB if ti % 2 == 0 else xsA

                for b in range(BL):
                    for j in range(2):
                        nc.sync.dma_start(
                            xs[b][j][65:97, :],
                            xT[t, :, b * NP + j * 512: b * NP + (j + 1) * 512])
                        nc.sync.dma_start(
                            xcb[b][j][65:97, :],
                            xT[t, :, b * NP + j * 512: b * NP + (j + 1) * 512])

                # ---- gate: hl = xs @ Wg, j-merged [128,1024] tiles -------
                # one tile per batch element: same element throughput as the
                # per-(b,j) split but half the instruction count, so the
                # per-instruction access/dispatch overheads and semaphore
                # hops halve on ScalarE/DVE
                hl, ygs = {}, {}
                for b in range(BL):
                    ph = psA.tile([128, 1024], fp32, tag=f"A{b}",
                                  name=f"ps_hl{b}_{ti}")
                    for j2 in range(2):
                        for k in range(4):
                            nc.tensor.matmul(
                                ph[:, j2 * 512 + k * 128:
                                   j2 * 512 + (k + 1) * 128],
                                xs[b][j2][0:97, k * 128:(k + 1) * 128],
                                wlin_g[:],
                            )
                    hlh = wpool.tile([128, 1024], bf16, tag=f"hl_g{b}",
                                     name=f"hl{b}_{ti}")
                    nc.scalar.copy(hlh[:], ph[:])
                    hl[b] = hlh
                pp_t = {}
                for b in range(BL):
                    pp = psB.tile([128, 1024], fp32, tag=f"B{b}",
                                  name=f"ps_pre{b}_{ti}")
                    for j in range(2):
                        nc.tensor.matmul(pp[:, j * 512:(j + 1) * 512],
                                         wres_g[:], xs[b][j][0:97, :],
                                         start=True, stop=False)
                    pp_t[b] = pp
                for b in range(BL):
                    yg = psA.tile([128, 1024], fp32, tag=f"A{b}",
                                  name=f"ps_yg{b}_{ti}")
                    for ls in range(NCHUNK):
                        nc.tensor.matmul(
                            yg[:, 0:128], vrt[:, ls, :],
                            hl[b][:, ls * 128:(ls + 1) * 128],
                            start=(ls == 0), stop=(ls == NCHUNK - 1),
                        )
                    ys = wpool.tile([128, 128], bf16, tag=f"yg{b}",
                                    name=f"yg{b}_{ti}")
                    nc.vector.tensor_copy(ys[:], yg[:, 0:128])
                    ygs[b] = ys

                zr, nm_t = {}, {}

                def emit_gate_chain(b):
                    pp = pp_t[b]
                    for j in range(2):
                        nc.tensor.matmul(pp[:, j * 512:(j + 1) * 512],
                                         ygs[b][:],
                                         urt[:, j * 512:(j + 1) * 512],
                                         start=False, stop=True)
                    pre = wpool.tile([128, 1024], bf16, tag=f"pre_g{b}",
                                     name=f"pre{b}_{ti}")
                    nc.vector.tensor_scalar_max(pre[:], pp[:], 0.0)
                    pc = psA.tile([128, 1024], fp32, tag=f"A{b}",
                                  name=f"ps_cent{b}_{ti}")
                    for j in range(2):
                        nc.tensor.matmul(pc[:, j * 512:(j + 1) * 512],
                                         pg[:], pre[:, j * 512:(j + 1) * 512])
                    sq = wpool.tile([128, 1024], bf16, tag=f"sq_g{b}",
                                    name=f"sq{b}_{ti}")
                    nc.scalar.activation(sq[:], pc[:], AF.Square)
                    pv = psB.tile([128, 1024], fp32, tag=f"B{b}",
                                  name=f"ps_var{b}_{ti}")
                    for j in range(2):
                        nc.tensor.matmul(pv[:, j * 512:(j + 1) * 512],
                                         gg[:], sq[:, j * 512:(j + 1) * 512])
                    lnv = wpool.tile([128, 1024], fp32, tag=f"lnv{b}",
                                     bufs=1, name=f"lnv{b}_{ti}")
                    nc.scalar.activation(lnv[:], pv[:], AF.Ln, bias=epsv[:])
                    rstd = wpool.tile([128, 1024], bf16, tag=f"rstd{b}",
                                      name=f"rstd{b}_{ti}")
                    nc.scalar.activation(rstd[:], lnv[:], AF.Exp, scale=-0.5)
                    nm = wpool.tile([128, 1024], bf16, tag=f"nm{b}",
                                    name=f"nm{b}_{ti}")
                    nc.vector.tensor_mul(nm[:], pc[:], rstd[:])
                    nm_t[b] = nm

                emit_gate_chain(0)
                emit_gate_chain(1)
                # sigmoid via exp + fast reciprocal (one act table):
                # z = 1 / (1 + exp(-x))
                for b in range(BL):
                    ug = wpool.tile([128, 1024], bf16, tag=f"ug{b}",
                                    name=f"ug{b}_{ti}")
                    nc.scalar.activation(ug[:], nm_t[b][:], AF.Exp,
                                         scale=-1.0)
                    dg = wpool.tile([128, 1024], fp32, tag=f"dg{b}",
                                    name=f"dg{b}_{ti}")
                    nc.vector.tensor_scalar_add(dg[:], ug[:], 1.0)
                    zrb = wpool.tile([128, 1024], fp32, tag=f"zr{b}",
                                     name=f"zr{b}_{ti}")
                    nc.vector.reciprocal_approx_fast(zrb[:], dg[:])
                    zr[b] = zrb
                    for j in range(2):
                        nc.vector.tensor_mul(xcb[b][j][0:64, :],
                                             zrb[0:64, j * 512:(j + 1) * 512],
                                             xs[b][j][0:64, :])

                # ---- candidate: b-stacked, j-merged ----------------------
                phc = psA.tile([128, 1024], fp32, tag="A0",
                               name=f"ps_hlc_{ti}")
                for j2 in range(2):
                    for k in range(4):
                        for b in range(BL):
                            nc.tensor.matmul(
                                phc[:, j2 * 512 + k * 128 + b * 64:
                                    j2 * 512 + k * 128 + (b + 1) * 64],
                                xcb[b][j2][0:97, k * 128:(k + 1) * 128],
                                wlin_c[:],
                            )
                hlcs = wpool.tile([128, 1024], bf16, tag="hl_c",
                                  name=f"hlc_{ti}")
                nc.scalar.copy(hlcs[:], phc[:])
                ppc = psB.tile([128, 1024], fp32, tag="B0",
                               name=f"ps_prec_{ti}")
                for j in range(2):
                    for b in range(BL):
                        nc.tensor.matmul(
                            ppc[b * 64:(b + 1) * 64, j * 512:(j + 1) * 512],
                            wres_c[:], xcb[b][j][0:97, :],
                            start=True, stop=False,
                            tile_position=(0, b * 64),
                        )
                yc = psA.tile([128, 1024], fp32, tag="A0",
                              name=f"ps_yc_{ti}")
                for ls in range(NCHUNK):
                    nc.tensor.matmul(
                        yc[:, 0:128], vrt[:, ls, :],
                        hlcs[:, ls * 128:(ls + 1) * 128],
                        start=(ls == 0), stop=(ls == NCHUNK - 1),
                    )
                ycs = wpool.tile([128, 128], bf16, tag="yc",
                                 name=f"yc_{ti}")
                nc.vector.tensor_copy(ycs[:], yc[:, 0:128])
                for j in range(2):
                    nc.tensor.matmul(ppc[:, j * 512:(j + 1) * 512], ycs[:],
                                     urt[:, j * 512:(j + 1) * 512],
                                     start=False, stop=True)

                prec = wpool.tile([128, 1024], bf16, tag="pre_c",
                                  name=f"prec_{ti}")
                nc.vector.tensor_scalar_max(prec[:], ppc[:], 0.0)
                pcc = psA.tile([128, 1024], fp32, tag="A1",
                               name=f"ps_centc_{ti}")
                for j in range(2):
                    nc.tensor.matmul(pcc[:, j * 512:(j + 1) * 512], pcb[:],
                                     prec[:, j * 512:(j + 1) * 512])
                sqc = wpool.tile([128, 1024], bf16, tag="sq_c",
                                 name=f"sqc_{ti}")
                nc.scalar.activation(sqc[:], pcc[:], AF.Square)
                pvc = psB.tile([128, 1024], fp32, tag="B1",
                               name=f"ps_varc_{ti}")
                for j in range(2):
                    nc.tensor.matmul(pvc[:, j * 512:(j + 1) * 512], gcb[:],
                                     sqc[:, j * 512:(j + 1) * 512])
                lnvc = wpool.tile([128, 1024], fp32, tag="lnvc",
                                  bufs=1, name=f"lnvc_{ti}")
                nc.scalar.activation(lnvc[:], pvc[:], AF.Ln, bias=epsv[:])
                rstdc = wpool.tile([128, 1024], bf16, tag="rstdc",
                                   name=f"rstdc_{ti}")
                nc.scalar.activation(rstdc[:], lnvc[:], AF.Exp, scale=-0.5)
                nmc = wpool.tile([128, 1024], bf16, tag="nmc",
                                 name=f"nmc_{ti}")
                nc.vector.tensor_mul(nmc[:], pcc[:], rstdc[:])
                # tanh via exp + fast reciprocal: 2 / (1 + exp(-2x)) - 1
                wc = wpool.tile([128, 1024], bf16, tag="wc",
                                name=f"wc_{ti}")
                nc.scalar.activation(wc[:], nmc[:], AF.Exp, scale=-2.0)
                dc = wpool.tile([128, 1024], fp32, tag="dc",
                                name=f"dc_{ti}")
                nc.vector.tensor_scalar_add(dc[:], wc[:], 1.0)
                rc = wpool.tile([128, 1024], fp32, tag="rc",
                                name=f"rc_{ti}")
                nc.vector.reciprocal_approx_fast(rc[:], dc[:])
                hcsm = wpool.tile([128, 1024], bf16, tag="hcs",
                                  name=f"hcs_{ti}")
                nc.vector.tensor_scalar(hcsm[:], rc[:], 2.0, 1.0,
                                        op0=mybir.AluOpType.mult,
                                        op1=mybir.AluOpType.subtract)

                # ---- state update + conv, per (b, j) ---------------------
                po = {}
                for b in range(BL):
                    po[b] = psA.tile([128, 1024], fp32, tag=f"A{b}",
                                     name=f"ps_out{b}_{ti}")
                for j in range(2):
                    for b in range(BL):
                        sl = slice(j * 512, (j + 1) * 512)
                        difb = wpool.tile([128, 512], bf16, tag=f"difb{b}{j}",
                                          name=f"difb{b}{j}_{ti}")
                        if b == 0:
                            nc.vector.tensor_sub(difb[64:128, :],
                                                 hcsm[0:64, sl],
                                                 xs[b][j][0:64, :])
                        else:
                            hc = wpool.tile([64, 512], bf16, tag=f"hc{b}{j}",
                                            name=f"hc{b}{j}_{ti}")
                            nc.vector.tensor_copy(hc[:], hcsm[64:128, sl])
                            nc.vector.tensor_sub(difb[64:128, :], hc[:],
                                                 xs[b][j][0:64, :])
                        zd = wpool.tile([64, 512], bf16, tag=f"zd{b}{j}",
                                        name=f"zd{b}{j}_{ti}")
                        nc.vector.tensor_mul(zd[:], zr[b][64:128, sl],
                                             difb[64:128, :])
                        nc.vector.tensor_add(xs_next[b][j][0:64, :],
                                             xs[b][j][0:64, :], zd[:])
                        nc.tensor.matmul(po[b][0:64, sl], convw[:],
                                         xs_next[b][j][0:65, :])
                for b in range(BL):
                    otb = wpool.tile([64, 1024], bf16, tag=f"ot{b}",
                                     name=f"ot{b}_{ti}")
                    nc.scalar.copy(otb[:], po[b][0:64, :])
                    nc.sync.dma_start(out_d[t, :, b * NP:(b + 1) * NP],
                                      otb[:])

    # populate .instr bytes for extended-inst InstISA subclasses (the
    # custom-DVE reciprocal) — without this walrus sees empty .instr and
    # fails with "ISA wrong length"
    mybir.codegen_inst_isa_subclasses(nc)
    return nc



# --------------------------------------------------------------------------
# entry point
# --------------------------------------------------------------------------

def kernel(**inputs):
    from concourse.bass_utils import run_bass_kernel_spmd  # noqa: PLC0415

    consts = _host_prep(inputs)
    xshards = _host_x(inputs)

    beta_g_nonzero = bool(np.any(consts["ln_g_b"] != 0))
    beta_c_nonzero = bool(np.any(consts["ln_c_b"] != 0))
    assert not beta_g_nonzero and not beta_c_nonzero, "beta path not wired yet"

    nc = _build_bass(beta_g_nonzero, beta_c_nonzero)

    base = {k: np.asarray(v) for k, v in consts.items()
            if k not in ("ln_g_b", "ln_c_b")}
    in_maps = []
    reps = int(os.environ.get("WORK_REPS", "1"))
    for c in range(NCORES):
        m = dict(base)
        m["xT"] = xshards[c]
        m["rep_tag"] = np.zeros((1, 8 * reps), np.float32)
        in_maps.append(m)

    res = run_bass_kernel_spmd(nc, in_maps, core_ids=list(range(NCORES)))
    outs = []
    for c in range(NCORES):
        o = np.asarray(res.results[c]["out"]).astype(np.float32)
        o = o.reshape(T, OUT, BL, NP)[:, :, :, :N]   # (T, 64, 2, 1000)
        outs.append(o.transpose(2, 3, 1, 0))         # (2, 1000, 64, 12)
    full = np.concatenate(outs, axis=0).astype(np.float32)
    return full


if __name__ == "__main__":
    print("kernel module loaded")



# revision 41
# speedup vs baseline: 1.3388x; 1.3388x over previous
"""AHGCRU (hypergraph-conv GRU) Trainium2 kernel.

Data-parallel over batch: B=16 -> 2 batch elements per NeuronCore (8 cores),
graph/params replicated, no collectives.

Host precompute collapses the N->M->N hypergraph aggregation into one dense
(N,N) matrix A2 = diag(Dinv) @ adj @ diag(Binv) @ S.T  (adjacency depends only
on nodevec/edgevec, so it is identical for every batch element and timestep),
then rank-128-truncates it via SVD (A2 ~= Ur @ Vr): the spectral tail is far
below the bf16 noise floor, and the factored aggregation
pre += Ur @ (Vr @ hl) costs 4x fewer PE columns than the dense product.

Device layout: channels-on-partitions, nodes on the free dim, both local batch
elements side by side: (C, 2048) = [b0 nodes 0:1024 | b1 nodes 1024:2048]
(N=1000 padded to 1024).  LayerNorm over channels is done with PE assists:
  - mean-centering via a projection-matrix GEMM  P = diag(gamma) @ (I - 1/C)
  - variance via a weighted ones-matrix GEMM     G[k,m] = (1/C)/gamma_k^2
  - rstd = exp(-0.5*ln(var+eps)) on ScalarE (ln/exp share one act table).
Sigmoid/tanh are computed as exp (ScalarE, same natural_log_exp table -> no
ACT_TABLE_LOAD per step) + fast custom-DVE reciprocal; relu runs on DVE
(tensor_scalar_max) so the ScalarE chain is ln/exp/square/copies only.
PSUM tiles are [128,1024] double banks keyed per batch element (tags
A0/A1/B0/B1 rotating through hl/y/cent/out and pre/var roles): with the
rank-128 aggregation the per-step matmul work is small, so the LN chains
run j-merged at 1024 wide -- half the instruction count and semaphore
traffic of a per-(b,j) split at identical element throughput.
State channels sit at partitions 0:64, a constant ones-row at 64 (also the
bias row of the bias-augmented 1x1 conv), and x_t channels at 65:97.
"""

import os
import sys

import numpy as np

for _p in ("/opt/trn_rl_repo", "/opt/pypackages"):
    if os.path.isdir(_p) and _p not in sys.path:
        sys.path.insert(0, _p)

B, N, F_IN, T = 16, 1000, 32, 12
HID = 64
OUT = 64
M = 500
EMB = 16
EPS = 1e-5

NCORES = 8
BL = B // NCORES          # 2 batch elements per core
NP = 1024                 # padded node count
NC = BL * NP              # 2048 free columns
NCHUNK = NP // 128        # 8 source chunks of 128 nodes per batch element


# --------------------------------------------------------------------------
# host-side preprocessing
# --------------------------------------------------------------------------

def _to_bf16(a):
    import ml_dtypes  # noqa: PLC0415

    return np.asarray(a, dtype=np.float32).astype(ml_dtypes.bfloat16)


def _host_prep(inputs):
    """Build all device-side constant tensors (shared across cores)."""
    f64 = np.float64
    nodevec = inputs["nodevec"].astype(f64)
    edgevec = inputs["edgevec"].astype(f64)

    DE = np.tanh(2.0 * nodevec)                     # (N, EMB)
    EE = np.tanh(2.0 * edgevec)                     # (M, EMB)
    adj = np.maximum(np.tanh(2.0 * (DE @ EE.T)), 0.0)   # (N, M)
    S = (adj > 0).astype(f64)
    Bsum = S.sum(0)
    Binv = np.where(Bsum > 0, 1.0 / np.maximum(Bsum, 1e-30), 0.0)   # (M,)
    Dsum = adj.sum(1)
    Dinv = np.where(Dsum > 0, 1.0 / np.maximum(Dsum, 1e-30), 0.0)   # (N,)

    A2 = (Dinv[:, None] * adj * Binv[None, :]) @ S.T     # (N, N): out = A2 @ hl
    # rank-R factorization A2 ~= Ur @ Vr: the SVD tail is far below the
    # bf16 noise floor (rel err 6.598e-3 at R=128 vs 6.580e-3 dense), and
    # the factored aggregation costs 48R output columns vs 24576 dense
    R = int(os.environ.get("A2_RANK", "128"))
    _u, _s, _vt = np.linalg.svd(A2)
    Ur = _u[:, :R] * _s[:R]                              # (N, R)
    Vr = _vt[:R]                                         # (R, N)
    VrT = np.zeros((NP, R), f64)
    VrT[:N] = Vr.T
    VrT = np.ascontiguousarray(VrT).reshape(NCHUNK, 128, R)
    UrT = np.zeros((R, NP), f64)
    UrT[:, :N] = Ur.T

    # channel reorder: xs rows were [xt 0:32 | state 32:96]; device buffer
    # uses [state 0:64 | ones 64 | xt 65:97] so the ones row also serves the
    # bias-augmented 1x1 conv (rhs rows 0:65)
    def reorder_aug(w, b_res, b_lin):
        # (96, C) weights -> (97, C) with the bias row at position 64
        w = np.asarray(w, f64)
        bias = np.asarray(b_res, f64) + np.asarray(b_lin, f64)
        return np.concatenate([w[32:96], bias[None, :], w[0:32]], axis=0)

    def reorder_zero(w):
        # (96, C) weights -> (97, C) with a zero row at position 64
        w = np.asarray(w, f64)
        z = np.zeros((1, w.shape[1]), f64)
        return np.concatenate([w[32:96], z, w[0:32]], axis=0)

    # gate output channels reordered to [r | z] so that r sits at partition
    # base 0 (walrus requires equal start partitions for DVE SB inputs)
    zperm = np.concatenate([np.arange(64, 128), np.arange(0, 64)])
    w_lin_g = reorder_zero(inputs["w_lin_g"])[:, zperm]       # (97, 128)
    wr_g = reorder_aug(inputs["w_res_g"], inputs["b_res_g"],
                       inputs["b_lin_g"])[:, zperm]
    w_lin_c = reorder_zero(inputs["w_lin_c"])                 # (97, 64)
    wr_c = reorder_aug(inputs["w_res_c"], inputs["b_res_c"], inputs["b_lin_c"])
    convw_aug = np.concatenate(
        [np.asarray(inputs["conv_w"], f64),
         np.asarray(inputs["conv_b"], f64)[None, :]], axis=0)  # (65, 64)

    def center_mats(gamma, C):
        g = np.asarray(gamma, f64)
        P = np.diag(g) @ (np.eye(C) - np.ones((C, C)) / C)    # cent = P @ pre
        gsq = np.where(g != 0, g * g, 1.0)
        w = (1.0 / C) / gsq                                   # var weights
        G = np.repeat(w[:, None], C, axis=1)                  # (C, C)
        return P, G

    Pg, Gg = center_mats(np.asarray(inputs["ln_g_w"], f64)[zperm], 2 * HID)
    Pc, Gc = center_mats(inputs["ln_c_w"], HID)               # (64, 64)

    def blockdiag(Ab, Bb):
        Z = np.zeros((Ab.shape[0] + Bb.shape[0], Ab.shape[1] + Bb.shape[1]), f64)
        Z[: Ab.shape[0], : Ab.shape[1]] = Ab
        Z[Ab.shape[0]:, Ab.shape[1]:] = Bb
        return Z

    Pcb = blockdiag(Pc, Pc)                                   # (128, 128)
    Gcb = blockdiag(Gc, Gc)

    consts = {
        "vrt": _to_bf16(VrT),                 # (8, 128, 128)
        "urt": _to_bf16(UrT),                 # (128, 1024)
        "wlin_g": _to_bf16(w_lin_g),          # (97, 128)
        "wres_g": _to_bf16(wr_g),             # (97, 128)
        "wlin_c": _to_bf16(w_lin_c),          # (97, 64)
        "wres_c": _to_bf16(wr_c),             # (97, 64)
        # lhsT for cent = P @ pre must be P.T (out = lhsT.T @ rhs)
        "pg": _to_bf16(Pg.T),                 # (128, 128)
        "gg": _to_bf16(Gg),                   # (128, 128) symmetric-by-rows
        "pcb": _to_bf16(Pcb.T),               # (128, 128)
        "gcb": _to_bf16(Gcb),
        "convw": _to_bf16(convw_aug),         # (65, 64)
        "ln_g_b": np.asarray(inputs["ln_g_b"], np.float32)[zperm],
        "ln_c_b": np.asarray(inputs["ln_c_b"], np.float32),
    }
    return consts


def _host_x(inputs):
    """x (B, N, F_IN, T) -> per-core (T, 32, 2048) bf16, channel-transposed."""
    x = np.asarray(inputs["x"], np.float32)
    xt = x.transpose(3, 2, 0, 1)                      # (T, F_IN, B, N)
    xp = np.zeros((T, F_IN, B, NP), np.float32)
    xp[:, :, :, :N] = xt
    shards = []
    for c in range(NCORES):
        sl = xp[:, :, c * BL:(c + 1) * BL, :].reshape(T, F_IN, NC)
        shards.append(_to_bf16(sl))
    return shards


# --------------------------------------------------------------------------
# device program
# --------------------------------------------------------------------------

def _patch_tile_drain():
    """walrus in this toolchain rejects >~2 sync-waits on one instruction;
    Tile's kernel-tail drain accumulates one wait per dangling semaphore.
    Split them across single-wait nofuse nops on the sync engine."""
    import concourse.mybir as mybir  # noqa: PLC0415
    from concourse.tile import TileContext  # noqa: PLC0415
    from concourse.vector_clock import ScopedClock  # noqa: PLC0415

    if getattr(TileContext, "_drain_waits_patched", False):
        return

    def _drain_and_barrier(self, tick_clock, wait_clock):
        collector = self.nc.sync.nop(nofuse=True, hint="tail_wait_0")
        wait_clock.add_sem_waits(
            collector.ins, ScopedClock({None: tick_clock.global_clock})
        )
        si = collector.ins.sync_info
        waits = list(si.on_wait) if si and si.on_wait else []
        if len(waits) > 1:
            collector.ins.sync_info = mybir.SyncInfo(
                on_wait=[waits[0]], on_update=list(si.on_update or [])
            )
            for k, w in enumerate(waits[1:]):
                extra = self.nc.sync.nop(nofuse=True, hint=f"tail_wait_{k + 1}")
                extra.ins.sync_info = mybir.SyncInfo(on_wait=[w], on_update=[])
        self.nc.sync.drain()
        self.nc.all_engine_barrier()
        popped = self.nc._tile_sem_poison_stack.pop()
        assert popped is self._sem_poison
        self.nc.clear_and_free_semaphores(list(self.sems.allocated().values()))
        self.nc.all_engine_barrier()

    TileContext._drain_and_barrier = _drain_and_barrier

    # Split >MAX_WAITS sem-waits on any scheduled instruction onto preceding
    # nofuse nops on the same engine (same-engine program order preserves
    # the wait semantics exactly).
    MAX_WAITS = int(os.environ.get("BASS_MAX_INST_WAITS", "1"))
    orig_lower = TileContext._lower_ordered_insts

    def _lower_ordered_insts(self, ordered):
        for bb_name, insts in ordered.items():
            out = []
            for inst in insts:
                si = inst.sync_info
                waits = list(si.on_wait) if si and si.on_wait else []
                if len(waits) > MAX_WAITS:
                    excess = waits[:-MAX_WAITS]
                    keep = waits[-MAX_WAITS:]
                    for j in range(0, len(excess), MAX_WAITS):
                        nop = mybir.InstNoOp(
                            name=self.nc.get_next_instruction_name(),
                            ins=[], outs=[], engine=inst.engine,
                        )
                        nop.bass_nofuse = True
                        nop.sync_info = mybir.SyncInfo(
                            on_wait=excess[j:j + MAX_WAITS], on_update=[]
                        )
                        out.append(nop)
                    inst.sync_info = mybir.SyncInfo(
                        on_wait=keep, on_update=list(si.on_update or [])
                    )
                out.append(inst)
            insts[:] = out
        return orig_lower(self, ordered)

    TileContext._lower_ordered_insts = _lower_ordered_insts
    TileContext._drain_waits_patched = True


def _build_bass(beta_g_nonzero, beta_c_nonzero):
    import concourse.bass as bass  # noqa: PLC0415
    import concourse.mybir as mybir  # noqa: PLC0415
    from concourse.tile import TileContext  # noqa: PLC0415

    _patch_tile_drain()

    fp32 = mybir.dt.float32
    bf16 = mybir.dt.bfloat16
    AF = mybir.ActivationFunctionType

    nc = bass.Bass()

    _reps = int(os.environ.get("WORK_REPS", "1"))
    rep_tag = nc.declare_dram_parameter("rep_tag", [1, 8 * _reps], fp32,
                                        isOutput=False)
    xT = nc.declare_dram_parameter("xT", [T, F_IN, NC], bf16, isOutput=False)
    vrt_d = nc.declare_dram_parameter("vrt", [NCHUNK, 128, 128], bf16, isOutput=False)
    urt_d = nc.declare_dram_parameter("urt", [128, NP], bf16, isOutput=False)
    wlin_g_d = nc.declare_dram_parameter("wlin_g", [97, 128], bf16, isOutput=False)
    wres_g_d = nc.declare_dram_parameter("wres_g", [97, 128], bf16, isOutput=False)
    wlin_c_d = nc.declare_dram_parameter("wlin_c", [97, 64], bf16, isOutput=False)
    wres_c_d = nc.declare_dram_parameter("wres_c", [97, 64], bf16, isOutput=False)
    pg_d = nc.declare_dram_parameter("pg", [128, 128], bf16, isOutput=False)
    gg_d = nc.declare_dram_parameter("gg", [128, 128], bf16, isOutput=False)
    pcb_d = nc.declare_dram_parameter("pcb", [128, 128], bf16, isOutput=False)
    gcb_d = nc.declare_dram_parameter("gcb", [128, 128], bf16, isOutput=False)
    convw_d = nc.declare_dram_parameter("convw", [65, 64], bf16, isOutput=False)
    out_d = nc.declare_dram_parameter("out", [T, OUT, NC], bf16, isOutput=True)

    with TileContext(nc) as tc:
        with (
            tc.tile_pool(name="const", bufs=1) as cpool,
            tc.tile_pool(name="state", bufs=1) as spool,
            tc.tile_pool(name="work", bufs=2) as wpool,
            tc.tile_pool(name="psA", bufs=1, space="PSUM") as psA,
            tc.tile_pool(name="psB", bufs=1, space="PSUM") as psB,
        ):
            # ---- constants into SBUF -------------------------------------
            vrt = cpool.tile([128, NCHUNK, 128], bf16, tag="vrt")
            for s in range(NCHUNK):
                nc.sync.dma_start(vrt[:, s, :], vrt_d[s])
            urt = cpool.tile([128, NP], bf16, tag="urt")
            nc.sync.dma_start(urt[:], urt_d[:])
            wlin_g = cpool.tile([97, 128], bf16, tag="wlg")
            nc.sync.dma_start(wlin_g[:], wlin_g_d[:])
            wres_g = cpool.tile([97, 128], bf16, tag="wrg")
            nc.sync.dma_start(wres_g[:], wres_g_d[:])
            wlin_c = cpool.tile([97, 64], bf16, tag="wlc")
            nc.sync.dma_start(wlin_c[:], wlin_c_d[:])
            wres_c = cpool.tile([97, 64], bf16, tag="wrc")
            nc.sync.dma_start(wres_c[:], wres_c_d[:])
            pg = cpool.tile([128, 128], bf16, tag="pg")
            nc.sync.dma_start(pg[:], pg_d[:])
            gg = cpool.tile([128, 128], bf16, tag="gg")
            nc.sync.dma_start(gg[:], gg_d[:])
            pcb = cpool.tile([128, 128], bf16, tag="pcb")
            nc.sync.dma_start(pcb[:], pcb_d[:])
            gcb = cpool.tile([128, 128], bf16, tag="gcb")
            nc.sync.dma_start(gcb[:], gcb_d[:])
            convw = cpool.tile([65, 64], bf16, tag="convw")
            nc.sync.dma_start(convw[:], convw_d[:])
            epsv = cpool.tile([128, 1], fp32, tag="epsv")
            nc.vector.memset(epsv[:], EPS)
            rtag = cpool.tile([1, 8 * _reps], fp32, tag="rtag")
            nc.sync.dma_start(rtag[:], rep_tag[:])

            # ---- persistent state buffers --------------------------------
            # per-(b, j) chunk tiles, 512 node cols each, so dependencies are
            # tracked at chunk granularity and the serial chain pipelines
            xsA = [[spool.tile([97, 512], bf16, tag=f"xsA{b}{j}",
                               name=f"xsA{b}{j}") for j in range(2)]
                   for b in range(BL)]
            xsB = [[spool.tile([97, 512], bf16, tag=f"xsB{b}{j}",
                               name=f"xsB{b}{j}") for j in range(2)]
                   for b in range(BL)]
            xcb = [[spool.tile([97, 512], bf16, tag=f"xc{b}{j}",
                               name=f"xcb{b}{j}") for j in range(2)]
                   for b in range(BL)]
            for b in range(BL):
                for j in range(2):
                    nc.vector.memset(xsA[b][j][64:65, :], 1.0)
                    nc.vector.memset(xsB[b][j][64:65, :], 1.0)
                    nc.vector.memset(xcb[b][j][64:65, :], 1.0)
                    nc.vector.memset(xsA[b][j][0:64, :], 0.0)   # h_0 = 0

            for b in range(BL):
                for j in range(2):
                    nc.vector.memset(xsB[b][j][0:64, :], 0.0)

            WORK_REPS = int(os.environ.get("WORK_REPS", "1"))

            for ti, t in enumerate([tt % T for tt in range(T * WORK_REPS)]):
                xs = xsA if ti % 2 == 0 else xsB
                xs_next = xsB if ti % 2 == 0 else xsA

                for b in range(BL):
                    for j in range(2):
                        nc.sync.dma_start(
                            xs[b][j][65:97, :],
                            xT[t, :, b * NP + j * 512: b * NP + (j + 1) * 512])
                        nc.sync.dma_start(
                            xcb[b][j][65:97, :],
                            xT[t, :, b * NP + j * 512: b * NP + (j + 1) * 512])

                # ---- gate: hl = xs @ Wg, j-merged [128,1024] tiles -------
                # one tile per batch element: same element throughput as the
                # per-(b,j) split but half the instruction count, so the
                # per-instruction access/dispatch overheads and semaphore
                # hops halve on ScalarE/DVE
                hl, ygs = {}, {}
                for b in range(BL):
                    ph = psA.tile([128, 1024], fp32, tag=f"A{b}",
                                  name=f"ps_hl{b}_{ti}")
                    for j2 in range(2):
                        for k in range(4):
                            nc.tensor.matmul(
                                ph[:, j2 * 512 + k * 128:
                                   j2 * 512 + (k + 1) * 128],
                                xs[b][j2][0:97, k * 128:(k + 1) * 128],
                                wlin_g[:],
                            )
                    hlh = wpool.tile([128, 1024], bf16, tag=f"hl_g{b}",
                                     name=f"hl{b}_{ti}")
                    nc.scalar.copy(hlh[:], ph[:])
                    hl[b] = hlh
                pp_t = {}
                for b in range(BL):
                    pp = psB.tile([128, 1024], fp32, tag=f"B{b}",
                                  name=f"ps_pre{b}_{ti}")
                    for j in range(2):
                        nc.tensor.matmul(pp[:, j * 512:(j + 1) * 512],
                                         wres_g[:], xs[b][j][0:97, :],
                                         start=True, stop=False)
                    pp_t[b] = pp
                for b in range(BL):
                    yg = psA.tile([128, 1024], fp32, tag=f"A{b}",
                                  name=f"ps_yg{b}_{ti}")
                    for ls in range(NCHUNK):
                        nc.tensor.matmul(
                            yg[:, 0:128], vrt[:, ls, :],
                            hl[b][:, ls * 128:(ls + 1) * 128],
                            start=(ls == 0), stop=(ls == NCHUNK - 1),
                        )
                    ys = wpool.tile([128, 128], bf16, tag=f"yg{b}",
                                    name=f"yg{b}_{ti}")
                    nc.vector.tensor_copy(ys[:], yg[:, 0:128])
                    ygs[b] = ys

                zr, nm_t = {}, {}

                def emit_gate_chain(b):
                    pp = pp_t[b]
                    for j in range(2):
                        nc.tensor.matmul(pp[:, j * 512:(j + 1) * 512],
                                         ygs[b][:],
                                         urt[:, j * 512:(j + 1) * 512],
                                         start=False, stop=True)
                    pre = wpool.tile([128, 1024], bf16, tag=f"pre_g{b}",
                                     name=f"pre{b}_{ti}")
                    nc.vector.tensor_scalar_max(pre[:], pp[:], 0.0)
                    pc = psA.tile([128, 1024], fp32, tag=f"A{b}",
                                  name=f"ps_cent{b}_{ti}")
                    for j in range(2):
                        nc.tensor.matmul(pc[:, j * 512:(j + 1) * 512],
                                         pg[:], pre[:, j * 512:(j + 1) * 512])
                    sq = wpool.tile([128, 1024], bf16, tag=f"sq_g{b}",
                                    name=f"sq{b}_{ti}")
                    nc.scalar.activation(sq[:], pc[:], AF.Square)
                    pv = psB.tile([128, 1024], fp32, tag=f"B{b}",
                                  name=f"ps_var{b}_{ti}")
                    for j in range(2):
                        nc.tensor.matmul(pv[:, j * 512:(j + 1) * 512],
                                         gg[:], sq[:, j * 512:(j + 1) * 512])
                    lnv = wpool.tile([128, 1024], fp32, tag=f"lnv{b}",
                                     bufs=1, name=f"lnv{b}_{ti}")
                    nc.scalar.activation(lnv[:], pv[:], AF.Ln, bias=epsv[:])
                    rstd = wpool.tile([128, 1024], bf16, tag=f"rstd{b}",
                                      name=f"rstd{b}_{ti}")
                    nc.scalar.activation(rstd[:], lnv[:], AF.Exp, scale=-0.5)
                    nm = wpool.tile([128, 1024], bf16, tag=f"nm{b}",
                                    name=f"nm{b}_{ti}")
                    nc.vector.tensor_mul(nm[:], pc[:], rstd[:])
                    nm_t[b] = nm

                emit_gate_chain(0)
                emit_gate_chain(1)
                # sigmoid via exp + fast reciprocal (one act table):
                # z = 1 / (1 + exp(-x))
                for b in range(BL):
                    ug = wpool.tile([128, 1024], bf16, tag=f"ug{b}",
                                    name=f"ug{b}_{ti}")
                    nc.scalar.activation(ug[:], nm_t[b][:], AF.Exp,
                                         scale=-1.0)
                    dg = wpool.tile([128, 1024], fp32, tag=f"dg{b}",
                                    name=f"dg{b}_{ti}")
                    nc.vector.tensor_scalar_add(dg[:], ug[:], 1.0)
                    zrb = wpool.tile([128, 1024], fp32, tag=f"zr{b}",
                                     name=f"zr{b}_{ti}")
                    nc.vector.reciprocal_approx_fast(zrb[:], dg[:])
                    zr[b] = zrb
                    for j in range(2):
                        nc.vector.tensor_mul(xcb[b][j][0:64, :],
                                             zrb[0:64, j * 512:(j + 1) * 512],
                                             xs[b][j][0:64, :])

                # ---- candidate: b-stacked, j-merged ----------------------
                phc = psA.tile([128, 1024], fp32, tag="A0",
                               name=f"ps_hlc_{ti}")
                for j2 in range(2):
                    for k in range(4):
                        for b in range(BL):
                            nc.tensor.matmul(
                                phc[:, j2 * 512 + k * 128 + b * 64:
                                    j2 * 512 + k * 128 + (b + 1) * 64],
                                xcb[b][j2][0:97, k * 128:(k + 1) * 128],
                                wlin_c[:],
                            )
                hlcs = wpool.tile([128, 1024], bf16, tag="hl_c",
                                  name=f"hlc_{ti}")
                nc.scalar.copy(hlcs[:], phc[:])
                ppc = psB.tile([128, 1024], fp32, tag="B0",
                               name=f"ps_prec_{ti}")
                for j in range(2):
                    for b in range(BL):
                        nc.tensor.matmul(
                            ppc[b * 64:(b + 1) * 64, j * 512:(j + 1) * 512],
                            wres_c[:], xcb[b][j][0:97, :],
                            start=True, stop=False,
                            tile_position=(0, b * 64),
                        )
                yc = psA.tile([128, 1024], fp32, tag="A0",
                              name=f"ps_yc_{ti}")
                for ls in range(NCHUNK):
                    nc.tensor.matmul(
                        yc[:, 0:128], vrt[:, ls, :],
                        hlcs[:, ls * 128:(ls + 1) * 128],
                        start=(ls == 0), stop=(ls == NCHUNK - 1),
                    )
                ycs = wpool.tile([128, 128], bf16, tag="yc",
                                 name=f"yc_{ti}")
                nc.vector.tensor_copy(ycs[:], yc[:, 0:128])
                for j in range(2):
                    nc.tensor.matmul(ppc[:, j * 512:(j + 1) * 512], ycs[:],
                                     urt[:, j * 512:(j + 1) * 512],
                                     start=False, stop=True)

                prec = wpool.tile([128, 1024], bf16, tag="pre_c",
                                  name=f"prec_{ti}")
                nc.vector.tensor_scalar_max(prec[:], ppc[:], 0.0)
                pcc = psA.tile([128, 1024], fp32, tag="A1",
                               name=f"ps_centc_{ti}")
                for j in range(2):
                    nc.tensor.matmul(pcc[:, j * 512:(j + 1) * 512], pcb[:],
                                     prec[:, j * 512:(j + 1) * 512])
                sqc = wpool.tile([128, 1024], bf16, tag="sq_c",
                                 name=f"sqc_{ti}")
                nc.scalar.activation(sqc[:], pcc[:], AF.Square)
                pvc = psB.tile([128, 1024], fp32, tag="B1",
                               name=f"ps_varc_{ti}")
                for j in range(2):
                    nc.tensor.matmul(pvc[:, j * 512:(j + 1) * 512], gcb[:],
                                     sqc[:, j * 512:(j + 1) * 512])
                lnvc = wpool.tile([128, 1024], fp32, tag="lnvc",
                                  bufs=1, name=f"lnvc_{ti}")
                nc.scalar.activation(lnvc[:], pvc[:], AF.Ln, bias=epsv[:])
                rstdc = wpool.tile([128, 1024], bf16, tag="rstdc",
                                   name=f"rstdc_{ti}")
                nc.scalar.activation(rstdc[:], lnvc[:], AF.Exp, scale=-0.5)
                nmc = wpool.tile([128, 1024], bf16, tag="nmc",
                                 name=f"nmc_{ti}")
                nc.vector.tensor_mul(nmc[:], pcc[:], rstdc[:])
                # tanh via exp + fast reciprocal: 2 / (1 + exp(-2x)) - 1
                wc = wpool.tile([128, 1024], bf16, tag="wc",
                                name=f"wc_{ti}")
                nc.scalar.activation(wc[:], nmc[:], AF.Exp, scale=-2.0)
                dc = wpool.tile([128, 1024], fp32, tag="dc",
                                name=f"dc_{ti}")
                nc.vector.tensor_scalar_add(dc[:], wc[:], 1.0)
                rc = wpool.tile([128, 1024], fp32, tag="rc",
                                name=f"rc_{ti}")
                nc.vector.reciprocal_approx_fast(rc[:], dc[:])
                hcsm = wpool.tile([128, 1024], bf16, tag="hcs",
                                  name=f"hcs_{ti}")
                nc.vector.tensor_scalar(hcsm[:], rc[:], 2.0, 1.0,
                                        op0=mybir.AluOpType.mult,
                                        op1=mybir.AluOpType.subtract)

                # ---- state update + conv, per (b, j) ---------------------
                po = {}
                for b in range(BL):
                    po[b] = psA.tile([128, 1024], fp32, tag=f"A{b}",
                                     name=f"ps_out{b}_{ti}")
                for j in range(2):
                    for b in range(BL):
                        sl = slice(j * 512, (j + 1) * 512)
                        difb = wpool.tile([128, 512], bf16, tag=f"difb{b}{j}",
                                          name=f"difb{b}{j}_{ti}")
                        if b == 0:
                            nc.vector.tensor_sub(difb[64:128, :],
                                                 hcsm[0:64, sl],
                                                 xs[b][j][0:64, :])
                        else:
                            hc = wpool.tile([64, 512], bf16, tag=f"hc{b}{j}",
                                            name=f"hc{b}{j}_{ti}")
                            nc.vector.tensor_copy(hc[:], hcsm[64:128, sl])
                            nc.vector.tensor_sub(difb[64:128, :], hc[:],
                                                 xs[b][j][0:64, :])
                        zd = wpool.tile([64, 512], bf16, tag=f"zd{b}{j}",
                                        name=f"zd{b}{j}_{ti}")
                        nc.vector.tensor_mul(zd[:], zr[b][64:128, sl],
                                             difb[64:128, :])
                        nc.vector.tensor_add(xs_next[b][j][0:64, :],
                                             xs[b][j][0:64, :], zd[:])
                        nc.tensor.matmul(po[b][0:64, sl], convw[:],
                                         xs_next[b][j][0:65, :])
                for b in range(BL):
                    otb = wpool.tile([64, 1024], bf16, tag=f"ot{b}",
                                     name=f"ot{b}_{ti}")
                    nc.scalar.copy(otb[:], po[b][0:64, :])
                    nc.sync.dma_start(out_d[t, :, b * NP:(b + 1) * NP],
                                      otb[:])

    # populate .instr bytes for extended-inst InstISA subclasses (the
    # custom-DVE reciprocal) — without this walrus sees empty .instr and
    # fails with "ISA wrong length"
    mybir.codegen_inst_isa_subclasses(nc)
    return nc



# --------------------------------------------------------------------------
# entry point
# --------------------------------------------------------------------------

def kernel(**inputs):
    from concourse.bass_utils import run_bass_kernel_spmd  # noqa: PLC0415

    consts = _host_prep(inputs)
    xshards = _host_x(inputs)

    beta_g_nonzero = bool(np.any(consts["ln_g_b"] != 0))
    beta_c_nonzero = bool(np.any(consts["ln_c_b"] != 0))
    assert not beta_g_nonzero and not beta_c_nonzero, "beta path not wired yet"

    nc = _build_bass(beta_g_nonzero, beta_c_nonzero)

    base = {k: np.asarray(v) for k, v in consts.items()
            if k not in ("ln_g_b", "ln_c_b")}
    in_maps = []
    reps = int(os.environ.get("WORK_REPS", "1"))
    for c in range(NCORES):
        m = dict(base)
        m["xT"] = xshards[c]
        m["rep_tag"] = np.zeros((1, 8 * reps), np.float32)
        in_maps.append(m)

    res = run_bass_kernel_spmd(nc, in_maps, core_ids=list(range(NCORES)))
    outs = []
    for c in range(NCORES):
        o = np.asarray(res.results[c]["out"]).astype(np.float32)
        o = o.reshape(T, OUT, BL, NP)[:, :, :, :N]   # (T, 64, 2, 1000)
        outs.append(o.transpose(2, 3, 1, 0))         # (2, 1000, 64, 12)
    full = np.concatenate(outs, axis=0).astype(np.float32)
    return full


if __name__ == "__main__":
    print("kernel module loaded")

